# revision 1
# baseline (speedup 1.0000x reference)
"""Trainium2 Bass kernel for nn_BoxDetectionLoss (8-core data parallel).

Math: reference loss = sum_{a,r,c}[ has_match ? coord+conf_loss : conf^2 ] / denom.
A pixel (r,c) can only match a target box t if r==tb[t,0] and c==tb[t,1]
(T=16 boxes per image), so the dense term is sum sigmoid(conf_ch)^2 over
channels {2,5,8}; the match term is a correction at <=16 pixels x 3 anchors
from 144 gathered elements per image.

Each of the 8 cores handles one batch image (pure data parallel).  Layout:
  - the 3 conf channels stream as whole-channel DMAs (8KB contiguous per
    partition x 128 partitions - the only descriptor shape that runs a DMA
    ring at full rate) spread over 3 rings: sync HWDGE carries ch0 +
    ch1-lower (+ the tiny const block at its tail), gpsimd SWDGE streams
    ch2 whole (early start, no ACT-table-load delay), and the scalar HWDGE
    ring gets only ch1-upper.  ch2 is the compute-tail channel.
  - per channel: ACT sigmoid (f32 -> bf16 out); ch0/ch1 squares are DVE
    tensor_tensor in bf16 (2x column rate) and the otherwise-idle PE
    reduces them with ones-vector matmuls accumulated into one PSUM
    [1,512] bank; the tail channel splits its square between ACT
    Square+accum_out and a DVE tt+reduce piece, sigmoid issued in two
    column pieces so DVE starts early.
  - the 144-element SWDGE indirect gather ships straight to HBM ("gout");
    the host computes the tiny match-correction term from it while the
    device streams (host also precomputes gather offsets / dup-keep mask).
  - final: PE collapses the [128,1] ACC partials to a scalar so the output
    DMA is a single descriptor (a [128]-wide store costs ~8us in receipt
    latency); host sums 8 cores' (out[2], corr) partials.
"""

import os

import numpy as np

B, C, H, W = 8, 9, 512, 512
T = 16
N_CORES = 8
CONF_CH = (2, 5, 8)
DENOM = float(B * H * W * 3)
MAGIC = 12582912.0  # 1.5 * 2^23: x+MAGIC-MAGIC rounds to nearest-even int

SPL = int(os.environ.get("SPL", "768"))  # ch2 square split: ACT cols [rest:], DVE cols [0:2048-SPL]
CORR = os.environ.get("CORR", "1") == "1"

# packed f32 constants: [T, 16] = tbf(4) | tp | keep | offs(9, exact ints) | pad
CST_COLS = 16


def make_cst(tb_i, tp_i):
    cst = np.zeros((T, CST_COLS), dtype=np.float32)
    cst[:, 0:4] = tb_i.astype(np.float32)
    cst[:, 4] = tp_i
    for t in range(T):
        dup = any((tb_i[t] == tb_i[t2]).all() for t2 in range(t))
        cst[t, 5] = 0.0 if dup else 1.0
    base = tb_i[:, 0].astype(np.int64) * W + tb_i[:, 1]
    offs = base[:, None] + np.arange(C, dtype=np.int64)[None, :] * (H * W)
    cst[:, 6:15] = offs.astype(np.float32)  # < 2^24, exact in f32
    return cst


_PROG = None


def _build_correction(nc, sp, ACC, ccol, bass, mybir, CST, G):
    f32 = mybir.dt.float32
    ALU = mybir.AluOpType
    ACT_F = mybir.ActivationFunctionType

    TBf = CST[:, 0:4]
    TP = CST[:, 4:5]
    KEEP = CST[:, 5:6]

    GS = sp.tile([T, C], f32)
    nc.scalar.activation(GS[:], G[:], ACT_F.Sigmoid)
    # channel ch = 3a + k: k=0 delta_r, k=1 delta_c, k=2 conf
    gs3 = GS[:].rearrange("p (a k) -> p k a", k=3)

    # pred = clip(tb + sigmoid*scale, 0, 511)
    predr = sp.tile([T, 3], f32)
    nc.vector.tensor_scalar(
        out=predr[:], in0=gs3[:, 0, :], scalar1=9.0, scalar2=TBf[:, 0:1],
        op0=ALU.mult, op1=ALU.add,
    )
    nc.vector.tensor_scalar(
        out=predr[:], in0=predr[:], scalar1=511.0, scalar2=0.0,
        op0=ALU.min, op1=ALU.max,
    )
    predc = sp.tile([T, 3], f32)
    nc.vector.tensor_scalar(
        out=predc[:], in0=gs3[:, 1, :], scalar1=16.0, scalar2=TBf[:, 1:2],
        op0=ALU.mult, op1=ALU.add,
    )
    nc.vector.tensor_scalar(
        out=predc[:], in0=predc[:], scalar1=511.0, scalar2=0.0,
        op0=ALU.min, op1=ALU.max,
    )

    # round-half-even: (x + 1.5*2^23) - 1.5*2^23
    rr = sp.tile([T, 3], f32)
    nc.vector.tensor_scalar(
        out=rr[:], in0=predr[:], scalar1=MAGIC, scalar2=None, op0=ALU.add
    )
    nc.vector.tensor_scalar(
        out=rr[:], in0=rr[:], scalar1=MAGIC, scalar2=None, op0=ALU.subtract
    )
    rc = sp.tile([T, 3], f32)
    nc.vector.tensor_scalar(
        out=rc[:], in0=predc[:], scalar1=MAGIC, scalar2=None, op0=ALU.add
    )
    nc.vector.tensor_scalar(
        out=rc[:], in0=rc[:], scalar1=MAGIC, scalar2=None, op0=ALU.subtract
    )

    # match mask: (rr==tb2) * (rc==tb3)
    m2 = sp.tile([T, 3], f32)
    nc.vector.tensor_scalar(
        out=m2[:], in0=rc[:], scalar1=TBf[:, 3:4], scalar2=None,
        op0=ALU.is_equal,
    )
    m = sp.tile([T, 3], f32)
    nc.vector.scalar_tensor_tensor(
        out=m[:], in0=rr[:], scalar=TBf[:, 2:3], in1=m2[:],
        op0=ALU.is_equal, op1=ALU.mult,
    )

    # coord = |predr-tb2| + |predc-tb3|; |x| as max(x, -x)
    d1 = sp.tile([T, 3], f32)
    nc.vector.tensor_scalar(
        out=d1[:], in0=predr[:], scalar1=TBf[:, 2:3], scalar2=None,
        op0=ALU.subtract,
    )
    d1n = sp.tile([T, 3], f32)
    nc.vector.tensor_scalar(
        out=d1n[:], in0=d1[:], scalar1=-1.0, scalar2=None, op0=ALU.mult
    )
    nc.vector.tensor_tensor(out=d1[:], in0=d1[:], in1=d1n[:], op=ALU.max)
    d2 = sp.tile([T, 3], f32)
    nc.vector.tensor_scalar(
        out=d2[:], in0=predc[:], scalar1=TBf[:, 3:4], scalar2=None,
        op0=ALU.subtract,
    )
    d2n = sp.tile([T, 3], f32)
    nc.vector.tensor_scalar(
        out=d2n[:], in0=d2[:], scalar1=-1.0, scalar2=None, op0=ALU.mult
    )
    nc.vector.tensor_tensor(out=d2[:], in0=d2[:], in1=d2n[:], op=ALU.max)
    # conf part tp*(tp-2*conf); total = d1 + (cf + d2)
    cf = sp.tile([T, 3], f32)
    nc.vector.tensor_scalar(
        out=cf[:], in0=gs3[:, 2, :], scalar1=-2.0, scalar2=TP[:],
        op0=ALU.mult, op1=ALU.add,
    )
    nc.vector.scalar_tensor_tensor(
        out=cf[:], in0=cf[:], scalar=TP[:], in1=d2[:],
        op0=ALU.mult, op1=ALU.add,
    )
    nc.vector.tensor_tensor(out=d1[:], in0=d1[:], in1=cf[:], op=ALU.add)
    # contribution = m * keep * total
    nc.vector.scalar_tensor_tensor(
        out=m[:], in0=m[:], scalar=KEEP[:], in1=d1[:],
        op0=ALU.mult, op1=ALU.mult,
    )
    nc.vector.tensor_reduce(
        out=ACC[0:T, ccol : ccol + 1], in_=m[:],
        axis=mybir.AxisListType.X, op=ALU.add,
    )


def _build_program(corr=CORR, spl=SPL):
    import concourse.bass as bass
    import concourse.tile as tile
    from concourse import bacc, mybir

    f32 = mybir.dt.float32
    bf16 = mybir.dt.bfloat16
    i32 = mybir.dt.int32
    ALU = mybir.AluOpType
    ACT_F = mybir.ActivationFunctionType

    nc = bacc.Bacc(
        "TRN2", target_bir_lowering=False, debug=False, num_devices=N_CORES
    )
    pol = nc.dram_tensor("pol", [C, H, W], f32, kind="ExternalInput").ap()
    cst = nc.dram_tensor("cst", [T, CST_COLS], f32, kind="ExternalInput").ap()
    out = nc.dram_tensor("out", [3], f32, kind="ExternalOutput").ap()
    gout = nc.dram_tensor("gout", [T, C], f32, kind="ExternalOutput").ap()

    with tile.TileContext(nc) as tc:
        with (
            tc.tile_pool(name="io", bufs=1) as io,
            tc.tile_pool(name="acc", bufs=1) as accp,
            tc.tile_pool(name="small", bufs=1) as sp,
            tc.tile_pool(name="psum", bufs=1, space="PSUM") as psum,
        ):
            # big tiles first so the DMA destinations stay 8KB-aligned
            # (misaligned dests get 4KB packets -> half ring rate)
            tins = []
            for k in range(3):
                tins.append(io.tile([128, 2048], f32, name=f"in{k}", tag=f"in{k}"))
            sigs = []
            for k in range(3):
                sigs.append(
                    io.tile([128, 2048], bf16, name=f"sg{k}", tag=f"sg{k}")
                )
            sqs = {}
            for k in (0, 1):
                sqs[k] = io.tile([128, 2048], bf16, name=f"sq{k}", tag=f"sq{k}")

            # ACC cols: 0 = ch2 ACT-square accum, 1 = ch2 DVE tail
            ACC = accp.tile([128, 2], f32)

            CSTt = sp.tile([T, CST_COLS], f32)

            # ---------- dense ladder: whole-channel DMAs (8KB lines x 128
            # partitions).  sync: ch0 then ch1; scalar ring (slow ~2us
            # start behind the ACT table loads): ch2 ----------
            views = [
                pol[ch].rearrange("(p a) w -> p (a w)", p=128) for ch in CONF_CH
            ]
            # sync: ch0, ch1-lower, then cst at the TAIL (tiny transfers at
            # a ring's head can degrade it; cst only feeds the host-side
            # correction gather, which just has to beat the final barrier);
            # qAct (starts late behind ACT table loads) gets only the small
            # ch1-upper; SWDGE (early start, no table delay) streams ch2 -
            # the tail channel - whole.
            nc.sync.dma_start(tins[0][:], views[0][:])
            nc.gpsimd.dma_start(tins[2][:], views[2][:])
            nc.scalar.dma_start(tins[1][64:128, :], views[1][64:128, :])
            nc.sync.dma_start(tins[1][0:64, :], views[1][0:64, :])
            nc.sync.dma_start(CSTt[:], cst[:])

            # gather offsets -> i32, SWDGE indirect gather, then ship the
            # 144 gathered values to HBM: the host computes the (tiny)
            # match-correction term while the dense stream runs on-device
            G = sp.tile([T, C], f32)
            if corr:
                OFFi = sp.tile([T, C], i32)
                nc.vector.tensor_copy(OFFi[:], CSTt[:, 6:15])
                nc.gpsimd.indirect_dma_start(
                    out=G[:], out_offset=None,
                    in_=pol.rearrange("c h (w a) -> (c h w) a", a=1),
                    in_offset=bass.IndirectOffsetOnAxis(ap=OFFi[:], axis=0),
                )
                nc.gpsimd.dma_start(gout[:], G[:])

            PACC0 = psum.tile([1, 512], f32, space="PSUM")
            PACC1 = psum.tile([1, 512], f32, space="PSUM")
            ONESB = sp.tile([128, 1], bf16)
            nc.vector.memset(ONESB[:], 1.0)
            ONESF = sp.tile([128, 1], f32)
            nc.vector.memset(ONESF[:], 1.0)
            OUTSB = sp.tile([1, 3], f32)

            # ch0 / ch1: sigmoid -> DVE bf16 square -> PE ones-matmul
            # reduce.  One PSUM bank per channel so each bank's DVE readout
            # runs as soon as that channel's matmuls retire (ch0's is fully
            # off the critical path).
            for k, pacc in ((0, PACC0), (1, PACC1)):
                nc.scalar.activation(sigs[k][:], tins[k][:], ACT_F.Sigmoid)
                nc.vector.tensor_tensor(
                    out=sqs[k][:], in0=sigs[k][:], in1=sigs[k][:], op=ALU.mult
                )
                for j in range(0, 2048, 512):
                    nc.tensor.matmul(
                        out=pacc[:], lhsT=ONESB[:], rhs=sqs[k][:, j : j + 512],
                        start=(j == 0), stop=(j == 1536),
                    )
                if k == 0:
                    nc.vector.tensor_reduce(
                        out=OUTSB[0:1, 1:2], in_=pacc[:],
                        axis=mybir.AxisListType.X, op=ALU.add,
                    )

            # ch2 (tail): sigmoid in 2 column pieces so DVE's square half
            # starts a piece earlier; ACT squares the later piece
            dve_end = 2048 - spl
            nc.scalar.activation(
                sigs[2][:, 0:dve_end], tins[2][:, 0:dve_end], ACT_F.Sigmoid
            )
            nc.scalar.activation(
                sigs[2][:, dve_end:], tins[2][:, dve_end:], ACT_F.Sigmoid
            )
            SQT = sp.tile([128, dve_end], bf16)
            nc.vector.tensor_tensor(
                out=SQT[:], in0=sigs[2][:, 0:dve_end], in1=sigs[2][:, 0:dve_end],
                op=ALU.mult,
            )
            nc.vector.tensor_reduce(
                out=ACC[:, 1:2], in_=SQT[:], axis=mybir.AxisListType.X,
                op=ALU.add,
            )
            nc.scalar.activation(
                tins[2][:, dve_end:], sigs[2][:, dve_end:], ACT_F.Square,
                accum_out=ACC[:, 0:1],
            )

            # ---------- final merge; PE collapses partitions so the out DMA
            # is a single descriptor (a [128]-wide store costs ~8us).
            # out[0] = partition-collapsed ACC sum, out[1] = PSUM col sums;
            # host adds the two. ----------
            nc.vector.tensor_reduce(
                out=OUTSB[0:1, 2:3], in_=PACC1[:], axis=mybir.AxisListType.X,
                op=ALU.add,
            )
            RED = sp.tile([128, 1], f32)
            nc.vector.tensor_reduce(
                out=RED[:], in_=ACC[:], axis=mybir.AxisListType.X, op=ALU.add
            )
            PS = psum.tile([1, 1], f32, space="PSUM")
            nc.tensor.matmul(out=PS[:], lhsT=RED[:], rhs=ONESF[:],
                             start=True, stop=True)
            nc.vector.tensor_copy(OUTSB[0:1, 0:1], PS[:])
            nc.sync.dma_start(out[:], OUTSB[:])

    nc.compile()
    return nc


def get_program():
    global _PROG
    if _PROG is None:
        _PROG = _build_program()
    return _PROG


def make_in_maps(policy_output, target_boxes, target_probs):
    policy_output = np.ascontiguousarray(np.asarray(policy_output, dtype=np.float32))
    target_boxes = np.ascontiguousarray(np.asarray(target_boxes, dtype=np.int32))
    target_probs = np.ascontiguousarray(np.asarray(target_probs, dtype=np.float32))
    assert policy_output.shape == (B, C, H, W)
    in_maps = []
    for i in range(N_CORES):
        in_maps.append(
            {
                "pol": policy_output[i],
                "cst": make_cst(target_boxes[i], target_probs[i]),
            }
        )
    return in_maps


def host_corr(g, tb_i, tp_i):
    """Match-term correction from the 144 gathered logits (f64, tiny)."""
    s = 1.0 / (1.0 + np.exp(-g.astype(np.float64)))  # [T, C]
    total = 0.0
    for t in range(T):
        if any((tb_i[t] == tb_i[t2]).all() for t2 in range(t)):
            continue  # an earlier identical box wins the match
        r, c, r2, c2 = (float(v) for v in tb_i[t])
        tp = float(tp_i[t])
        for a in range(3):
            pr = min(max(r + 9.0 * s[t, 3 * a + 0], 0.0), 511.0)
            pc = min(max(c + 16.0 * s[t, 3 * a + 1], 0.0), 511.0)
            if np.round(pr) == r2 and np.round(pc) == c2:
                conf = s[t, 3 * a + 2]
                total += abs(pr - r2) + abs(pc - c2) + tp * (tp - 2.0 * conf)
    return total


def kernel(policy_output, target_boxes, target_probs):
    from concourse.bass_utils import run_bass_kernel_spmd

    nc = get_program()
    tb = np.ascontiguousarray(np.asarray(target_boxes, dtype=np.int32))
    tp = np.ascontiguousarray(np.asarray(target_probs, dtype=np.float32))
    in_maps = make_in_maps(policy_output, target_boxes, target_probs)
    res = run_bass_kernel_spmd(nc, in_maps, list(range(N_CORES)))
    total = 0.0
    for i in range(N_CORES):
        total += float(res.results[i]["out"].sum(dtype=np.float64))
        total += host_corr(np.asarray(res.results[i]["gout"]), tb[i], tp[i])
    return np.float32(total / DENOM)



# revision 2
# speedup vs baseline: 1.0542x; 1.0542x over previous
"""Trainium2 Bass kernel for nn_BoxDetectionLoss (8-core data parallel).

Math: reference loss = sum_{a,r,c}[ has_match ? coord+conf_loss : conf^2 ] / denom.
A pixel (r,c) can only match a target box t if r==tb[t,0] and c==tb[t,1]
(T=16 boxes per image), so the dense term is sum sigmoid(conf_ch)^2 over
channels {2,5,8}; the match term is a tiny correction at <=16 pixels x 3
anchors (144 logits per image) computed ON HOST in f64 straight from the
full inputs kernel() already holds (bit-identical to a device gather).

Each of the 8 cores handles one batch image (pure data parallel).

Device pipeline (per core):
  - the 3 conf channels stream as column-chunked HWDGE DMAs so compute
    can start while the stream runs.  Chunks land in a deterministic
    order (SDMA serves qAct strictly before qSP on this part, and each
    ring is FIFO), so compute is emitted in expected landing order.
  - per chunk: ACT sigmoid (f32 -> bf16), DVE square (bf16 2x rate),
    PE ones-matmul accumulated into one PSUM [1,512] bank.  ACT is
    faster per byte than DMA, so compute hides under the stream; the
    last chunk is small to keep the tail thin.
  - final: DVE reduces the PSUM bank to a scalar; single-descriptor
    out DMA.  Host sums 8 cores' partials + correction, divides.
"""

import os

import numpy as np

B, C, H, W = 8, 9, 512, 512
T = 16
N_CORES = 8
CONF_CH = (2, 5, 8)
DENOM = float(B * H * W * 3)

# chunk plan: (ring, conf_idx, col0, cols); cols multiple of 512.
# ring "q" = scalar/qAct (served first), "s" = sync/qSP.
# Emitted compute order == listed order == expected landing order.
PLANS = {
    # all on the sync ring: guaranteed FIFO landing order
    "P1": [
        ("s", 0, 0, 512), ("s", 0, 512, 1024), ("s", 0, 1536, 512),
        ("s", 1, 0, 1024), ("s", 1, 1024, 1024),
        ("s", 2, 0, 1024), ("s", 2, 1024, 512), ("s", 2, 1536, 512),
    ],
    # early chunks on the scalar ring (strict-priority first), tail on sync
    "P2": [
        ("q", 0, 0, 512), ("q", 0, 512, 1024), ("q", 0, 1536, 512),
        ("s", 1, 0, 1024), ("s", 1, 1024, 1024),
        ("s", 2, 0, 1024), ("s", 2, 1024, 512), ("s", 2, 1536, 512),
    ],
    # alternate rings per chunk
    "P3": [
        ("q", 0, 0, 512), ("s", 0, 512, 1024), ("q", 0, 1536, 512),
        ("s", 1, 0, 1024), ("q", 1, 1024, 1024),
        ("s", 2, 0, 1024), ("q", 2, 1024, 512), ("s", 2, 1536, 512),
    ],
}
PLAN = os.environ.get("PLAN", "P2")

_PROG = None


def _build_program(plan=None):
    import concourse.bass as bass  # noqa: F401
    import concourse.tile as tile
    from concourse import bacc, mybir

    f32 = mybir.dt.float32
    bf16 = mybir.dt.bfloat16
    ALU = mybir.AluOpType
    ACT_F = mybir.ActivationFunctionType

    chunks = PLANS[PLAN] if plan is None else plan

    nc = bacc.Bacc(
        "TRN2", target_bir_lowering=False, debug=False, num_devices=N_CORES
    )
    pol = nc.dram_tensor("pol", [C, H, W], f32, kind="ExternalInput").ap()
    out = nc.dram_tensor("out", [1], f32, kind="ExternalOutput").ap()

    with tile.TileContext(nc) as tc:
        with (
            tc.tile_pool(name="io", bufs=1) as io,
            tc.tile_pool(name="small", bufs=1) as sp,
            tc.tile_pool(name="psum", bufs=1, space="PSUM") as psum,
        ):
            # big tiles first so DMA destinations stay well-aligned
            TIN = io.tile([128, 6144], f32, name="tin", tag="tin")
            SG = io.tile([128, 6144], bf16, name="sg", tag="sg")
            SQ = io.tile([128, 6144], bf16, name="sq", tag="sq")

            ONESB = sp.tile([128, 1], bf16)
            OUTSB = sp.tile([1, 1], f32)
            PACC = psum.tile([1, 512], f32, space="PSUM")

            views = [
                pol[ch].rearrange("(p a) w -> p (a w)", p=128) for ch in CONF_CH
            ]

            # ---- input DMAs, chunked; per-ring FIFO keeps landing order ----
            for ring, ci, c0, cols in chunks:
                g0 = ci * 2048 + c0
                eng = nc.scalar if ring == "q" else nc.sync
                eng.dma_start(
                    TIN[:, g0 : g0 + cols], views[ci][:, c0 : c0 + cols]
                )

            nc.vector.memset(ONESB[:], 1.0)

            # ---- pipelined compute in landing order ----
            nmm = sum(cols // 512 for _, _, _, cols in chunks)
            mm = 0
            for ring, ci, c0, cols in chunks:
                g0 = ci * 2048 + c0
                nc.scalar.activation(
                    SG[:, g0 : g0 + cols], TIN[:, g0 : g0 + cols], ACT_F.Sigmoid
                )
                nc.vector.tensor_tensor(
                    out=SQ[:, g0 : g0 + cols],
                    in0=SG[:, g0 : g0 + cols],
                    in1=SG[:, g0 : g0 + cols],
                    op=ALU.mult,
                )
                for j in range(g0, g0 + cols, 512):
                    nc.tensor.matmul(
                        out=PACC[:],
                        lhsT=ONESB[:],
                        rhs=SQ[:, j : j + 512],
                        start=(mm == 0),
                        stop=(mm == nmm - 1),
                    )
                    mm += 1

            # ---- final: PSUM row -> scalar -> single-descriptor store ----
            nc.vector.tensor_reduce(
                out=OUTSB[:], in_=PACC[:], axis=mybir.AxisListType.X, op=ALU.add
            )
            nc.sync.dma_start(out[:], OUTSB[:])

    nc.compile()
    return nc


def get_program():
    global _PROG
    if _PROG is None:
        _PROG = _build_program()
    return _PROG


def make_in_maps(policy_output, target_boxes=None, target_probs=None):
    policy_output = np.ascontiguousarray(
        np.asarray(policy_output, dtype=np.float32)
    )
    assert policy_output.shape == (B, C, H, W)
    return [{"pol": policy_output[i]} for i in range(N_CORES)]


def host_corr(pol_i, tb_i, tp_i):
    """Match-term correction (f64, <=48 anchors) from the full inputs.

    For each target box t and anchor a the corrected contribution replaces
    the dense fp term at that cell: coord + (conf-tp)^2 - conf^2
    = |pr-r2| + |pc-c2| + tp*(tp - 2*conf).
    """
    tbl = tb_i.astype(np.int64)
    g = pol_i[:, tbl[:, 0], tbl[:, 1]].astype(np.float64)  # [C, T]
    s = 1.0 / (1.0 + np.exp(-g))
    total = 0.0
    for t in range(T):
        if any((tbl[t] == tbl[t2]).all() for t2 in range(t)):
            continue  # an earlier identical box wins the match
        r, c, r2, c2 = (float(v) for v in tbl[t])
        tp = float(tp_i[t])
        for a in range(3):
            pr = min(max(r + 9.0 * s[3 * a + 0, t], 0.0), 511.0)
            pc = min(max(c + 16.0 * s[3 * a + 1, t], 0.0), 511.0)
            if np.round(pr) == r2 and np.round(pc) == c2:
                conf = s[3 * a + 2, t]
                total += abs(pr - r2) + abs(pc - c2) + tp * (tp - 2.0 * conf)
    return total


def kernel(policy_output, target_boxes, target_probs):
    from concourse.bass_utils import run_bass_kernel_spmd

    nc = get_program()
    pol = np.ascontiguousarray(np.asarray(policy_output, dtype=np.float32))
    tb = np.ascontiguousarray(np.asarray(target_boxes, dtype=np.int32))
    tp = np.ascontiguousarray(np.asarray(target_probs, dtype=np.float32))
    in_maps = make_in_maps(pol)
    res = run_bass_kernel_spmd(nc, in_maps, list(range(N_CORES)))
    total = 0.0
    for i in range(N_CORES):
        total += float(res.results[i]["out"].sum(dtype=np.float64))
        total += host_corr(pol[i], tb[i], tp[i])
    return np.float32(total / DENOM)


# revision 4
# speedup vs baseline: 1.0607x; 1.0062x over previous
"""Trainium2 Bass kernel for nn_BoxDetectionLoss (8-core data parallel).

Math: reference loss = sum_{a,r,c}[ has_match ? coord+conf_loss : conf^2 ] / denom.
A pixel (r,c) can only match a target box t if r==tb[t,0] and c==tb[t,1]
(T=16 boxes per image), so the dense term is sum sigmoid(conf_ch)^2 over
channels {2,5,8}; the match term is a tiny correction at <=16 pixels x 3
anchors (144 logits per image) computed ON HOST in f64 straight from the
full inputs kernel() already holds (bit-identical to a device gather).

Each of the 8 cores handles one batch image (pure data parallel).

Device pipeline (per core):
  - the 3 conf channels stream as column-chunked HWDGE DMAs so compute
    can start while the stream runs.  Chunks land in a deterministic
    order (SDMA serves qAct strictly before qSP on this part, and each
    ring is FIFO), so compute is emitted in expected landing order.
  - per chunk: ACT sigmoid (f32 -> bf16), DVE square (bf16 2x rate),
    PE ones-matmul accumulated into one PSUM [1,512] bank.  ACT is
    faster per byte than DMA, so compute hides under the stream; the
    last chunk is small to keep the tail thin.
  - final: DVE reduces the PSUM bank to a scalar; single-descriptor
    out DMA.  Host sums 8 cores' partials + correction, divides.
"""

import os

import numpy as np

B, C, H, W = 8, 9, 512, 512
T = 16
N_CORES = 8
CONF_CH = (2, 5, 8)
DENOM = float(B * H * W * 3)

# chunk plan: (ring, conf_idx, col0, cols); cols multiple of 512.
# ring "q" = scalar/qAct, "s" = sync/qSP.  A single ring drains FIFO at
# full aggregate rate (one HWDGE queue saturates all 16 SDMA engines),
# so landing order == issue order == emitted compute order.
PLANS = {
    # all on the sync ring: guaranteed FIFO landing order, 4KB lines mid
    "P4": [
        ("s", 0, 0, 512), ("s", 0, 512, 1024), ("s", 0, 1536, 512),
        ("s", 1, 0, 1024), ("s", 1, 1024, 1024),
        ("s", 2, 0, 1024), ("s", 2, 1024, 512), ("s", 2, 1536, 512),
    ],
}
PLAN = os.environ.get("PLAN", "P4")

_PROG = None


def _build_program(plan=None):
    import concourse.bass as bass  # noqa: F401
    import concourse.tile as tile
    from concourse import bacc, mybir

    f32 = mybir.dt.float32
    bf16 = mybir.dt.bfloat16
    ALU = mybir.AluOpType
    ACT_F = mybir.ActivationFunctionType

    chunks = PLANS[PLAN] if plan is None else plan

    nc = bacc.Bacc(
        "TRN2", target_bir_lowering=False, debug=False, num_devices=N_CORES
    )
    pol = nc.dram_tensor("pol", [C, H, W], f32, kind="ExternalInput").ap()
    out = nc.dram_tensor("out", [2], f32, kind="ExternalOutput").ap()

    with tile.TileContext(nc) as tc:
        with (
            tc.tile_pool(name="io", bufs=1) as io,
            tc.tile_pool(name="small", bufs=1) as sp,
            tc.tile_pool(name="psum", bufs=1, space="PSUM") as psum,
        ):
            # big tiles first so DMA destinations stay well-aligned
            TIN = io.tile([128, 6144], f32, name="tin", tag="tin")
            SG = io.tile([128, 6144], bf16, name="sg", tag="sg")
            SQ = io.tile([128, 6144], bf16, name="sq", tag="sq")

            ONESB = sp.tile([128, 1], bf16)
            OUTSB = sp.tile([1, 2], f32)
            # bank A: all chunks but the last (readout overlaps the tail);
            # bank B: the last chunk only
            PACC_A = psum.tile([1, 512], f32, space="PSUM")
            PACC_B = psum.tile([1, 512], f32, space="PSUM")

            views = [
                pol[ch].rearrange("(p a) w -> p (a w)", p=128) for ch in CONF_CH
            ]

            # ---- input DMAs, chunked; per-ring FIFO keeps landing order ----
            for ring, ci, c0, cols in chunks:
                g0 = ci * 2048 + c0
                eng = nc.scalar if ring == "q" else nc.sync
                eng.dma_start(
                    TIN[:, g0 : g0 + cols], views[ci][:, c0 : c0 + cols]
                )

            nc.vector.memset(ONESB[:], 1.0)

            # ---- pipelined compute in landing order ----
            nmm_a = sum(cols // 512 for _, _, _, cols in chunks[:-1])
            mm = 0
            for k, (ring, ci, c0, cols) in enumerate(chunks):
                last = k == len(chunks) - 1
                g0 = ci * 2048 + c0
                nc.scalar.activation(
                    SG[:, g0 : g0 + cols], TIN[:, g0 : g0 + cols], ACT_F.Sigmoid
                )
                nc.vector.tensor_tensor(
                    out=SQ[:, g0 : g0 + cols],
                    in0=SG[:, g0 : g0 + cols],
                    in1=SG[:, g0 : g0 + cols],
                    op=ALU.mult,
                )
                for jj, j in enumerate(range(g0, g0 + cols, 512)):
                    nc.tensor.matmul(
                        out=PACC_B[:, 0:cols] if last else PACC_A[:],
                        lhsT=ONESB[:],
                        rhs=SQ[:, j : j + 512],
                        start=(jj == 0) if last else (mm == 0),
                        stop=last or (mm == nmm_a - 1),
                    )
                    mm += 1
                if k == len(chunks) - 2:
                    # bank A complete: read it out while the tail chunk runs
                    nc.vector.tensor_reduce(
                        out=OUTSB[0:1, 0:1], in_=PACC_A[:],
                        axis=mybir.AxisListType.X, op=ALU.add,
                    )

            # ---- tail: bank B -> scalar; single 8B store; host adds ----
            nc.vector.tensor_reduce(
                out=OUTSB[0:1, 1:2], in_=PACC_B[:, 0 : chunks[-1][3]],
                axis=mybir.AxisListType.X, op=ALU.add,
            )
            nc.sync.dma_start(out[:], OUTSB[:])

    nc.compile()
    return nc


def get_program():
    global _PROG
    if _PROG is None:
        _PROG = _build_program()
    return _PROG


def make_in_maps(policy_output, target_boxes=None, target_probs=None):
    policy_output = np.ascontiguousarray(
        np.asarray(policy_output, dtype=np.float32)
    )
    assert policy_output.shape == (B, C, H, W)
    return [{"pol": policy_output[i]} for i in range(N_CORES)]


def host_corr(pol_i, tb_i, tp_i):
    """Match-term correction (f64, <=48 anchors) from the full inputs.

    For each target box t and anchor a the corrected contribution replaces
    the dense fp term at that cell: coord + (conf-tp)^2 - conf^2
    = |pr-r2| + |pc-c2| + tp*(tp - 2*conf).
    """
    tbl = tb_i.astype(np.int64)
    g = pol_i[:, tbl[:, 0], tbl[:, 1]].astype(np.float64)  # [C, T]
    s = 1.0 / (1.0 + np.exp(-g))
    total = 0.0
    for t in range(T):
        if any((tbl[t] == tbl[t2]).all() for t2 in range(t)):
            continue  # an earlier identical box wins the match
        r, c, r2, c2 = (float(v) for v in tbl[t])
        tp = float(tp_i[t])
        for a in range(3):
            pr = min(max(r + 9.0 * s[3 * a + 0, t], 0.0), 511.0)
            pc = min(max(c + 16.0 * s[3 * a + 1, t], 0.0), 511.0)
            if np.round(pr) == r2 and np.round(pc) == c2:
                conf = s[3 * a + 2, t]
                total += abs(pr - r2) + abs(pc - c2) + tp * (tp - 2.0 * conf)
    return total


def kernel(policy_output, target_boxes, target_probs):
    from concourse.bass_utils import run_bass_kernel_spmd

    nc = get_program()
    pol = np.ascontiguousarray(np.asarray(policy_output, dtype=np.float32))
    tb = np.ascontiguousarray(np.asarray(target_boxes, dtype=np.int32))
    tp = np.ascontiguousarray(np.asarray(target_probs, dtype=np.float32))
    in_maps = make_in_maps(pol)
    res = run_bass_kernel_spmd(nc, in_maps, list(range(N_CORES)))
    total = 0.0
    for i in range(N_CORES):
        total += float(res.results[i]["out"].sum(dtype=np.float64))
        total += host_corr(pol[i], tb[i], tp[i])
    return np.float32(total / DENOM)


# revision 6
# speedup vs baseline: 1.1293x; 1.0647x over previous
"""Trainium2 Bass kernel for nn_BoxDetectionLoss (8-core data parallel).

Math: reference loss = sum_{a,r,c}[ has_match ? coord+conf_loss : conf^2 ] / denom.
A pixel (r,c) can only match a target box t if r==tb[t,0] and c==tb[t,1]
(T=16 boxes per image), so the dense term is sum sigmoid(conf_ch)^2 over
channels {2,5,8}; the match term is a tiny correction at <=16 pixels x 3
anchors (144 logits per image) computed ON HOST in f64 straight from the
full inputs kernel() already holds (bit-identical to a device gather).

Each of the 8 cores handles one batch image (pure data parallel).

Device pipeline (per core):
  - the 3 conf channels stream as column-chunked HWDGE DMAs so compute
    can start while the stream runs.  Chunks land in a deterministic
    order (SDMA serves qAct strictly before qSP on this part, and each
    ring is FIFO), so compute is emitted in expected landing order.
  - per chunk: ACT sigmoid (f32 -> bf16), DVE square (bf16 2x rate),
    PE ones-matmul accumulated into one PSUM [1,512] bank.  ACT is
    faster per byte than DMA, so compute hides under the stream; the
    last chunk is small to keep the tail thin.
  - final: DVE reduces the PSUM bank to a scalar; single-descriptor
    out DMA.  Host sums 8 cores' partials + correction, divides.
"""

import os

import numpy as np

B, C, H, W = 8, 9, 512, 512
T = 16
N_CORES = 8
CONF_CH = (2, 5, 8)
DENOM = float(B * H * W * 3)

# chunk plan: (ring, conf_idx, col0, cols); cols multiple of 512.
# ring "q" = scalar/qAct, "s" = sync/qSP.  A single ring drains FIFO at
# full aggregate rate (one HWDGE queue saturates all 16 SDMA engines),
# so landing order == issue order == emitted compute order.  Chunks are
# big early (drain time must cover the ~0.7us/DMA doorbell cadence on
# the issuing engine) and small late (thin compute tail).
def _mkplan(sizes):
    plan, ci, c0 = [], 0, 0
    for s in sizes:
        plan.append(("s", ci, c0, s))
        c0 += s
        if c0 == 2048:
            ci, c0 = ci + 1, 0
    assert ci == 3 and c0 == 0
    return plan


PLANS = {
    "P5": _mkplan([1024, 1024, 1024, 1024, 1024, 512, 512]),
    "P6": _mkplan([2048, 1024, 1024, 1024, 512, 512]),
    "P7": _mkplan([1024, 1024, 1024, 1024, 512, 512, 512, 512]),
}
PLAN = os.environ.get("PLAN", "P5")

_PROG = None


def _build_program(plan=None):
    import concourse.bass as bass  # noqa: F401
    import concourse.tile as tile
    from concourse import bacc, mybir

    f32 = mybir.dt.float32
    bf16 = mybir.dt.bfloat16
    ALU = mybir.AluOpType
    ACT_F = mybir.ActivationFunctionType

    chunks = PLANS[PLAN] if plan is None else plan

    nc = bacc.Bacc(
        "TRN2", target_bir_lowering=False, debug=False, num_devices=N_CORES
    )
    pol = nc.dram_tensor("pol", [C, H, W], f32, kind="ExternalInput").ap()
    out = nc.dram_tensor("out", [1], f32, kind="ExternalOutput").ap()

    with tile.TileContext(nc) as tc:
        with (
            tc.tile_pool(name="io", bufs=1) as io,
            tc.tile_pool(name="small", bufs=1) as sp,
            tc.tile_pool(name="psum", bufs=1, space="PSUM") as psum,
        ):
            # big tiles first so DMA destinations stay well-aligned
            TIN = io.tile([128, 6144], f32, name="tin", tag="tin")
            SG = io.tile([128, 6144], bf16, name="sg", tag="sg")
            SQ = io.tile([128, 6144], bf16, name="sq", tag="sq")

            ONESB = sp.tile([128, 1], bf16)
            OUTSB = sp.tile([1, 1], f32)
            PACC = psum.tile([1, 512], f32, space="PSUM")

            views = [
                pol[ch].rearrange("(p a) w -> p (a w)", p=128) for ch in CONF_CH
            ]

            # ---- input DMAs, chunked; single-ring FIFO = landing order ----
            for ring, ci, c0, cols in chunks:
                g0 = ci * 2048 + c0
                eng = nc.scalar if ring == "q" else nc.sync
                eng.dma_start(
                    TIN[:, g0 : g0 + cols], views[ci][:, c0 : c0 + cols]
                )

            nc.vector.memset(ONESB[:], 1.0)

            # ---- pipelined compute in landing order ----
            nmm = sum(cols // 512 for _, _, _, cols in chunks)
            mm = 0
            for ring, ci, c0, cols in chunks:
                g0 = ci * 2048 + c0
                nc.scalar.activation(
                    SG[:, g0 : g0 + cols], TIN[:, g0 : g0 + cols], ACT_F.Sigmoid
                )
                nc.vector.tensor_tensor(
                    out=SQ[:, g0 : g0 + cols],
                    in0=SG[:, g0 : g0 + cols],
                    in1=SG[:, g0 : g0 + cols],
                    op=ALU.mult,
                )
                for j in range(g0, g0 + cols, 512):
                    nc.tensor.matmul(
                        out=PACC[:],
                        lhsT=ONESB[:],
                        rhs=SQ[:, j : j + 512],
                        start=(mm == 0),
                        stop=(mm == nmm - 1),
                    )
                    mm += 1

            # ---- tail: one PSUM-row reduce; single 4B store; host sums ----
            nc.vector.tensor_reduce(
                out=OUTSB[:], in_=PACC[:], axis=mybir.AxisListType.X, op=ALU.add
            )
            nc.sync.dma_start(out[:], OUTSB[:])

    nc.compile()
    return nc


def get_program():
    global _PROG
    if _PROG is None:
        _PROG = _build_program()
    return _PROG


def make_in_maps(policy_output, target_boxes=None, target_probs=None):
    policy_output = np.ascontiguousarray(
        np.asarray(policy_output, dtype=np.float32)
    )
    assert policy_output.shape == (B, C, H, W)
    return [{"pol": policy_output[i]} for i in range(N_CORES)]


def host_corr(pol_i, tb_i, tp_i):
    """Match-term correction (f64, <=48 anchors) from the full inputs.

    For each target box t and anchor a the corrected contribution replaces
    the dense fp term at that cell: coord + (conf-tp)^2 - conf^2
    = |pr-r2| + |pc-c2| + tp*(tp - 2*conf).
    """
    tbl = tb_i.astype(np.int64)
    g = pol_i[:, tbl[:, 0], tbl[:, 1]].astype(np.float64)  # [C, T]
    s = 1.0 / (1.0 + np.exp(-g))
    total = 0.0
    for t in range(T):
        if any((tbl[t] == tbl[t2]).all() for t2 in range(t)):
            continue  # an earlier identical box wins the match
        r, c, r2, c2 = (float(v) for v in tbl[t])
        tp = float(tp_i[t])
        for a in range(3):
            pr = min(max(r + 9.0 * s[3 * a + 0, t], 0.0), 511.0)
            pc = min(max(c + 16.0 * s[3 * a + 1, t], 0.0), 511.0)
            if np.round(pr) == r2 and np.round(pc) == c2:
                conf = s[3 * a + 2, t]
                total += abs(pr - r2) + abs(pc - c2) + tp * (tp - 2.0 * conf)
    return total


def kernel(policy_output, target_boxes, target_probs):
    from concourse.bass_utils import run_bass_kernel_spmd

    nc = get_program()
    pol = np.ascontiguousarray(np.asarray(policy_output, dtype=np.float32))
    tb = np.ascontiguousarray(np.asarray(target_boxes, dtype=np.int32))
    tp = np.ascontiguousarray(np.asarray(target_probs, dtype=np.float32))
    in_maps = make_in_maps(pol)
    res = run_bass_kernel_spmd(nc, in_maps, list(range(N_CORES)))
    total = 0.0
    for i in range(N_CORES):
        total += float(res.results[i]["out"].sum(dtype=np.float64))
        total += host_corr(pol[i], tb[i], tp[i])
    return np.float32(total / DENOM)


# revision 28
# speedup vs baseline: 1.2218x; 1.0819x over previous
"""Trainium2 Bass kernel for nn_BoxDetectionLoss (8-core data parallel).

Math: reference loss = sum_{a,r,c}[ has_match ? coord+conf_loss : conf^2 ] / denom.
A pixel (r,c) can only match a target box t if r==tb[t,0] and c==tb[t,1]
(T=16 boxes per image), so the dense term is sum sigmoid(conf_ch)^2 over
channels {2,5,8}; the match term is a tiny correction at <=16 pixels x 3
anchors (144 logits per image) computed ON HOST in f64 straight from the
full inputs kernel() already holds (bit-identical to a device gather).

Each of the 8 cores handles one batch image (pure data parallel).

Device pipeline (per core), plan P12 (~25.7us vs 29.9us baseline):
  - the 3 conf channels stream as column-chunked DMAs ping-ponged over
    the two HWDGE rings (qSP/qAct).  SDMA engines round-robin between
    rings at packet granularity, so per-chunk sem-flushes hide under the
    other ring's packets and the aggregate stays at the ~25 GB/s/engine
    packet rate (~400 GB/s burst, ~320 GB/s sustained incl ramp).
  - ring s carries 3584 cols vs ring q 2560, so ring q drains early and
    the final two chunks land SOLO and staggered -> thin compute tail.
  - per chunk: ACT sigmoid (f32 -> bf16), DVE square (bf16 2x rate), PE
    ones-matmul into one PSUM [1,512] bank.  ACT (1 elem/cycle/lane) is
    faster per byte than the stream, so compute hides under it.
  - tail chunk: ACT Square + accum_out (f32 row-sums, no DVE/ones-mm;
    Square shares the resident table sets) -> tiny PE partition-collapse
    matmul; the big PSUM reduce runs on DVE DURING the tail.
  - single 16B out store (1 descriptor; a [128]-wide store costs ~8us in
    receipt latency).  Host sums 8 cores' partials + correction, divides.
"""

import os

import numpy as np

B, C, H, W = 8, 9, 512, 512
T = 16
N_CORES = 8
CONF_CH = (2, 5, 8)
DENOM = float(B * H * W * 3)

# chunk plan: (ring, conf_idx, col0, cols); cols multiple of 512.
# ring "q" = scalar/qAct, "s" = sync/qSP.  A single ring drains FIFO at
# full aggregate rate (one HWDGE queue saturates all 16 SDMA engines),
# so landing order == issue order == emitted compute order.  Chunks are
# big early (drain time must cover the ~0.7us/DMA doorbell cadence on
# the issuing engine) and small late (thin compute tail).
def _mkplan(sizes):
    plan, ci, c0 = [], 0, 0
    for s in sizes:
        plan.append(("s", ci, c0, s))
        c0 += s
        if c0 == 2048:
            ci, c0 = ci + 1, 0
    assert ci == 3 and c0 == 0
    return plan


def _pingpong(sizes, rings=None, compute=None):
    # alternate rings per chunk: both HWDGE queues stay busy and each
    # ring's per-chunk sem-flush hides under the other ring's packets
    plan = _mkplan(sizes)
    if rings is None:
        rings = ["q" if i % 2 else "s" for i in range(len(plan))]
    chunks = [(r, ci, c0, cols)
              for r, (_, ci, c0, cols) in zip(rings, plan, strict=True)]
    return {"chunks": chunks,
            "compute": compute or list(range(len(chunks)))}


PLANS = {
    "P8": _pingpong([1024, 1024, 1024, 1024, 512, 512, 512, 512]),
    # ring s carries 3584 cols, ring q 2560: ring q drains first, so the
    # last two chunks (both ring s) land ALONE at the stream end instead
    # of as a simultaneous pair -> thinner ACT tail
    "P11": _pingpong(
        [1024, 1024, 1024, 1024, 512, 512, 512, 512],
        rings=["s", "q", "s", "q", "s", "q", "s", "s"],
    ),
    # same layout + ACT-Square/accum tail for the last chunk (no DVE
    # square, no ones-matmul, PSUM reduce overlaps the tail)
    "P12": dict(
        _pingpong(
            [1024, 1024, 1024, 1024, 512, 512, 512, 512],
            rings=["s", "q", "s", "q", "s", "q", "s", "s"],
        ),
        act_tail=True,
    ),
    # tapered tail: ring q (2560 cols) drains early, ring s (3584) lands
    # its last three chunks (512/256/256) solo and staggered
    "P13": dict(
        _pingpong(
            [1024, 1024, 1024, 1024, 512, 512, 512, 256, 256],
            rings=["s", "q", "s", "q", "s", "q", "s", "s", "s"],
        ),
        act_tail=True,
    ),
    # P12 with one fewer chunk (fewer doorbells / end events)
    "P14": dict(
        _pingpong(
            [1024, 1024, 1024, 1024, 1024, 512, 512],
            rings=["s", "q", "s", "q", "s", "s", "s"],
        ),
        act_tail=True,
    ),
}
PLAN = os.environ.get("PLAN", "P12")

_PROG = None


def _build_program(plan=None, bir_lowering=False):
    import concourse.bass as bass  # noqa: F401
    import concourse.tile as tile
    from concourse import bacc, mybir

    f32 = mybir.dt.float32
    bf16 = mybir.dt.bfloat16
    ALU = mybir.AluOpType
    ACT_F = mybir.ActivationFunctionType

    chunks = PLANS[PLAN] if plan is None else plan

    nc = bacc.Bacc(
        "TRN2", target_bir_lowering=bir_lowering, debug=False,
        num_devices=N_CORES
    )
    pol = nc.dram_tensor("pol", [C, H, W], f32, kind="ExternalInput").ap()
    out = nc.dram_tensor("out", [4], f32, kind="ExternalOutput").ap()

    chunk_list = chunks["chunks"]
    order = chunks["compute"]
    act_tail = chunks.get("act_tail", False)
    # bank A: 512-multiple chunks (ones-matmul slices [1,512]); bank B:
    # the small (<512) tail chunks, all the same width.  With act_tail,
    # the last chunk instead goes through ACT Square+accum.
    tail_k = order[-1] if act_tail else None
    a_idx = [k for k in order
             if chunk_list[k][3] % 512 == 0 and k != tail_k]
    b_idx = [k for k in order
             if chunk_list[k][3] % 512 != 0 and k != tail_k]
    b_cols = chunk_list[b_idx[0]][3] if b_idx else 0

    with tile.TileContext(nc) as tc:
        with (
            tc.tile_pool(name="io", bufs=1) as io,
            tc.tile_pool(name="small", bufs=1) as sp,
            tc.tile_pool(name="psum", bufs=1, space="PSUM") as psum,
        ):
            # big tiles first so DMA destinations stay well-aligned
            TIN = io.tile([128, 6144], f32, name="tin", tag="tin")
            SG = io.tile([128, 6144], bf16, name="sg", tag="sg")
            SQ = io.tile([128, 6144], bf16, name="sq", tag="sq")

            ONESB = sp.tile([128, 1], bf16)
            ONESF = sp.tile([128, 1], f32)
            ACC = sp.tile([128, 1], f32)
            OUTSB = sp.tile([1, 4], f32)
            PACC_A = psum.tile([1, 512], f32, space="PSUM")
            PACC_B = psum.tile([1, 512], f32, space="PSUM")
            PS = psum.tile([1, 1], f32, space="PSUM")

            views = [
                pol[ch].rearrange("(p a) w -> p (a w)", p=128) for ch in CONF_CH
            ]

            # ---- input DMAs, chunked; per-ring FIFO + packet-level ring
            # round-robin = deterministic landing order ----
            for ring, ci, c0, cols in chunk_list:
                g0 = ci * 2048 + c0
                eng = nc.scalar if ring == "q" else nc.sync
                eng.dma_start(
                    TIN[:, g0 : g0 + cols], views[ci][:, c0 : c0 + cols]
                )

            nc.vector.memset(ONESB[:], 1.0)
            if act_tail:
                nc.vector.memset(ONESF[:], 1.0)
            nc.vector.memset(OUTSB[:], 0.0)

            # ---- pipelined compute in landing order ----
            for k in order:
                ring, ci, c0, cols = chunk_list[k]
                g0 = ci * 2048 + c0
                nc.scalar.activation(
                    SG[:, g0 : g0 + cols], TIN[:, g0 : g0 + cols], ACT_F.Sigmoid
                )
                if k == tail_k:
                    # tail chunk: square+row-accumulate on ACT (second
                    # pass), collapse partitions with one tiny PE matmul
                    nc.scalar.activation(
                        SQ[:, g0 : g0 + cols],
                        SG[:, g0 : g0 + cols],
                        ACT_F.Square,
                        accum_out=ACC[:],
                    )
                    nc.tensor.matmul(
                        out=PS[:], lhsT=ACC[:], rhs=ONESF[:],
                        start=True, stop=True,
                    )
                    nc.vector.tensor_copy(OUTSB[0:1, 2:3], PS[:])
                    continue
                nc.vector.tensor_tensor(
                    out=SQ[:, g0 : g0 + cols],
                    in0=SG[:, g0 : g0 + cols],
                    in1=SG[:, g0 : g0 + cols],
                    op=ALU.mult,
                )
                if cols % 512 == 0:
                    for j in range(g0, g0 + cols, 512):
                        nc.tensor.matmul(
                            out=PACC_A[:],
                            lhsT=ONESB[:],
                            rhs=SQ[:, j : j + 512],
                            start=(k == a_idx[0] and j == g0),
                            stop=(k == a_idx[-1] and j == g0 + cols - 512),
                        )
                else:
                    nc.tensor.matmul(
                        out=PACC_B[:, 0:cols],
                        lhsT=ONESB[:],
                        rhs=SQ[:, g0 : g0 + cols],
                        start=(k == b_idx[0]),
                        stop=(k == b_idx[-1]),
                    )
                # emit bank readouts as soon as each bank completes so
                # they precede later tail work in the DVE engine FIFO
                if k == a_idx[-1]:
                    nc.vector.tensor_reduce(
                        out=OUTSB[0:1, 0:1], in_=PACC_A[:],
                        axis=mybir.AxisListType.X, op=ALU.add,
                    )
                if b_idx and k == b_idx[-1]:
                    nc.vector.tensor_reduce(
                        out=OUTSB[0:1, 1:2], in_=PACC_B[:, 0:b_cols],
                        axis=mybir.AxisListType.X, op=ALU.add,
                    )

            # ---- single 8B store; host sums the partials ----
            nc.sync.dma_start(out[:], OUTSB[:])

    nc.compile()
    return nc


def get_program():
    global _PROG
    if _PROG is None:
        _PROG = _build_program()
    return _PROG


def make_in_maps(policy_output, target_boxes=None, target_probs=None):
    policy_output = np.ascontiguousarray(
        np.asarray(policy_output, dtype=np.float32)
    )
    assert policy_output.shape == (B, C, H, W)
    return [{"pol": policy_output[i]} for i in range(N_CORES)]


def host_corr(pol_i, tb_i, tp_i):
    """Match-term correction (f64, <=48 anchors) from the full inputs.

    For each target box t and anchor a the corrected contribution replaces
    the dense fp term at that cell: coord + (conf-tp)^2 - conf^2
    = |pr-r2| + |pc-c2| + tp*(tp - 2*conf).
    """
    tbl = tb_i.astype(np.int64)
    g = pol_i[:, tbl[:, 0], tbl[:, 1]].astype(np.float64)  # [C, T]
    s = 1.0 / (1.0 + np.exp(-g))
    total = 0.0
    for t in range(T):
        if any((tbl[t] == tbl[t2]).all() for t2 in range(t)):
            continue  # an earlier identical box wins the match
        r, c, r2, c2 = (float(v) for v in tbl[t])
        tp = float(tp_i[t])
        for a in range(3):
            pr = min(max(r + 9.0 * s[3 * a + 0, t], 0.0), 511.0)
            pc = min(max(c + 16.0 * s[3 * a + 1, t], 0.0), 511.0)
            if np.round(pr) == r2 and np.round(pc) == c2:
                conf = s[3 * a + 2, t]
                total += abs(pr - r2) + abs(pc - c2) + tp * (tp - 2.0 * conf)
    return total


def kernel(policy_output, target_boxes, target_probs):
    from concourse.bass_utils import run_bass_kernel_spmd

    nc = get_program()
    pol = np.ascontiguousarray(np.asarray(policy_output, dtype=np.float32))
    tb = np.ascontiguousarray(np.asarray(target_boxes, dtype=np.int32))
    tp = np.ascontiguousarray(np.asarray(target_probs, dtype=np.float32))
    in_maps = make_in_maps(pol)
    res = run_bass_kernel_spmd(nc, in_maps, list(range(N_CORES)))
    total = 0.0
    for i in range(N_CORES):
        total += float(res.results[i]["out"].sum(dtype=np.float64))
        total += host_corr(pol[i], tb[i], tp[i])
    return np.float32(total / DENOM)


# revision 31
# speedup vs baseline: 1.2467x; 1.0204x over previous
"""Trainium2 Bass kernel for nn_BoxDetectionLoss (8-core data parallel).

Math: reference loss = sum_{a,r,c}[ has_match ? coord+conf_loss : conf^2 ] / denom.
A pixel (r,c) can only match a target box t if r==tb[t,0] and c==tb[t,1]
(T=16 boxes per image), so the dense term is sum sigmoid(conf_ch)^2 over
channels {2,5,8}; the match term is a tiny correction at <=16 pixels x 3
anchors (144 logits per image) computed ON HOST in f64 straight from the
full inputs kernel() already holds (bit-identical to a device gather).

Each of the 8 cores handles one batch image (pure data parallel).

Device pipeline (per core), plan P12 (~25.7us vs 29.9us baseline):
  - the 3 conf channels stream as column-chunked DMAs ping-ponged over
    the two HWDGE rings (qSP/qAct).  SDMA engines round-robin between
    rings at packet granularity, so per-chunk sem-flushes hide under the
    other ring's packets and the aggregate stays at the ~25 GB/s/engine
    packet rate (~400 GB/s burst, ~320 GB/s sustained incl ramp).
  - ring s carries 3584 cols vs ring q 2560, so ring q drains early and
    the final two chunks land SOLO and staggered -> thin compute tail.
  - per chunk: ACT sigmoid (f32 -> bf16), DVE square (bf16 2x rate), PE
    ones-matmul into one PSUM [1,512] bank.  ACT (1 elem/cycle/lane) is
    faster per byte than the stream, so compute hides under it.
  - tail chunk: ACT Square + accum_out (f32 row-sums, no DVE/ones-mm;
    Square shares the resident table sets) -> tiny PE partition-collapse
    matmul; the big PSUM reduce runs on DVE DURING the tail.
  - single 16B out store (1 descriptor; a [128]-wide store costs ~8us in
    receipt latency).  Host sums 8 cores' partials + correction, divides.
"""

import os

import numpy as np

B, C, H, W = 8, 9, 512, 512
T = 16
N_CORES = 8
CONF_CH = (2, 5, 8)
DENOM = float(B * H * W * 3)

# chunk plan: (ring, conf_idx, col0, cols); cols multiple of 512.
# ring "q" = scalar/qAct, "s" = sync/qSP.  A single ring drains FIFO at
# full aggregate rate (one HWDGE queue saturates all 16 SDMA engines),
# so landing order == issue order == emitted compute order.  Chunks are
# big early (drain time must cover the ~0.7us/DMA doorbell cadence on
# the issuing engine) and small late (thin compute tail).
def _mkplan(sizes):
    plan, ci, c0 = [], 0, 0
    for s in sizes:
        plan.append(("s", ci, c0, s))
        c0 += s
        if c0 == 2048:
            ci, c0 = ci + 1, 0
    assert ci == 3 and c0 == 0
    return plan


def _pingpong(sizes, rings=None, compute=None):
    # alternate rings per chunk: both HWDGE queues stay busy and each
    # ring's per-chunk sem-flush hides under the other ring's packets
    plan = _mkplan(sizes)
    if rings is None:
        rings = ["q" if i % 2 else "s" for i in range(len(plan))]
    chunks = [(r, ci, c0, cols)
              for r, (_, ci, c0, cols) in zip(rings, plan, strict=True)]
    return {"chunks": chunks,
            "compute": compute or list(range(len(chunks)))}


PLANS = {
    "P8": _pingpong([1024, 1024, 1024, 1024, 512, 512, 512, 512]),
    # ring s carries 3584 cols, ring q 2560: ring q drains first, so the
    # last two chunks (both ring s) land ALONE at the stream end instead
    # of as a simultaneous pair -> thinner ACT tail
    "P11": _pingpong(
        [1024, 1024, 1024, 1024, 512, 512, 512, 512],
        rings=["s", "q", "s", "q", "s", "q", "s", "s"],
    ),
    # same layout + ACT-Square/accum tail for the last chunk (no DVE
    # square, no ones-matmul, PSUM reduce overlaps the tail)
    "P12": dict(
        _pingpong(
            [1024, 1024, 1024, 1024, 512, 512, 512, 512],
            rings=["s", "q", "s", "q", "s", "q", "s", "s"],
        ),
        act_tail=True,
    ),
    # tapered tail: ring q (2560 cols) drains early, ring s (3584) lands
    # its last three chunks (512/256/256) solo and staggered
    "P13": dict(
        _pingpong(
            [1024, 1024, 1024, 1024, 512, 512, 512, 256, 256],
            rings=["s", "q", "s", "q", "s", "q", "s", "s", "s"],
        ),
        act_tail=True,
    ),
    # P12 with one fewer chunk (fewer doorbells / end events)
    "P14": dict(
        _pingpong(
            [1024, 1024, 1024, 1024, 1024, 512, 512],
            rings=["s", "q", "s", "q", "s", "s", "s"],
        ),
        act_tail=True,
    ),
    # 6 chunks: bigger leading pairs, same staggered-solo tail
    "P17": {
        "chunks": [
            ("s", 0, 0, 1536), ("q", 1, 0, 1536),
            ("s", 2, 0, 1024), ("q", 2, 1024, 1024),
            ("s", 0, 1536, 512), ("s", 1, 1536, 512),
        ],
        "compute": list(range(6)),
        "act_tail": True,
    },
}
PLAN = os.environ.get("PLAN", "P12")

_PROG = None


def _build_program(plan=None, bir_lowering=False):
    import concourse.bass as bass  # noqa: F401
    import concourse.tile as tile
    from concourse import bacc, mybir

    f32 = mybir.dt.float32
    bf16 = mybir.dt.bfloat16
    ALU = mybir.AluOpType
    ACT_F = mybir.ActivationFunctionType

    chunks = PLANS[PLAN] if plan is None else plan

    nc = bacc.Bacc(
        "TRN2", target_bir_lowering=bir_lowering, debug=False,
        num_devices=N_CORES
    )
    pol = nc.dram_tensor("pol", [C, H, W], f32, kind="ExternalInput").ap()
    out = nc.dram_tensor("out", [4], f32, kind="ExternalOutput").ap()

    chunk_list = chunks["chunks"]
    order = chunks["compute"]
    act_tail = chunks.get("act_tail", False)
    # bank A: 512-multiple chunks (ones-matmul slices [1,512]); bank B:
    # the small (<512) tail chunks, all the same width.  With act_tail,
    # the last chunk instead goes through ACT Square+accum.
    tail_k = order[-1] if act_tail else None
    a_idx = [k for k in order
             if chunk_list[k][3] % 512 == 0 and k != tail_k]
    b_idx = [k for k in order
             if chunk_list[k][3] % 512 != 0 and k != tail_k]
    b_cols = chunk_list[b_idx[0]][3] if b_idx else 0

    with tile.TileContext(nc) as tc:
        with (
            tc.tile_pool(name="io", bufs=1) as io,
            tc.tile_pool(name="small", bufs=1) as sp,
            tc.tile_pool(name="psum", bufs=1, space="PSUM") as psum,
        ):
            # big tiles first so DMA destinations stay well-aligned
            TIN = io.tile([128, 6144], f32, name="tin", tag="tin")
            SG = io.tile([128, 6144], bf16, name="sg", tag="sg")
            SQ = io.tile([128, 6144], bf16, name="sq", tag="sq")

            ONESB = sp.tile([128, 1], bf16)
            ONESF = sp.tile([128, 1], f32)
            ACC = sp.tile([128, 1], f32)
            OUTSB = sp.tile([1, 4], f32)
            PACC_A = psum.tile([1, 512], f32, space="PSUM")
            PACC_B = psum.tile([1, 512], f32, space="PSUM")
            PS = psum.tile([1, 1], f32, space="PSUM")

            views = [
                pol[ch].rearrange("(p a) w -> p (a w)", p=128) for ch in CONF_CH
            ]

            # ---- input DMAs, chunked; per-ring FIFO + packet-level ring
            # round-robin = deterministic landing order ----
            for ring, ci, c0, cols in chunk_list:
                g0 = ci * 2048 + c0
                eng = nc.scalar if ring == "q" else nc.sync
                eng.dma_start(
                    TIN[:, g0 : g0 + cols], views[ci][:, c0 : c0 + cols]
                )

            nc.vector.memset(ONESB[:], 1.0)
            if act_tail:
                nc.vector.memset(ONESF[:], 1.0)
            nc.vector.memset(OUTSB[:], 0.0)

            # ---- pipelined compute in landing order ----
            for k in order:
                ring, ci, c0, cols = chunk_list[k]
                g0 = ci * 2048 + c0
                nc.scalar.activation(
                    SG[:, g0 : g0 + cols], TIN[:, g0 : g0 + cols], ACT_F.Sigmoid
                )
                if k == tail_k:
                    # tail chunk: square+row-accumulate on ACT (second
                    # pass), collapse partitions with one tiny PE matmul
                    nc.scalar.activation(
                        SQ[:, g0 : g0 + cols],
                        SG[:, g0 : g0 + cols],
                        ACT_F.Square,
                        accum_out=ACC[:],
                    )
                    nc.tensor.matmul(
                        out=PS[:], lhsT=ACC[:], rhs=ONESF[:],
                        start=True, stop=True,
                    )
                    nc.vector.tensor_copy(OUTSB[0:1, 2:3], PS[:])
                    continue
                nc.vector.tensor_tensor(
                    out=SQ[:, g0 : g0 + cols],
                    in0=SG[:, g0 : g0 + cols],
                    in1=SG[:, g0 : g0 + cols],
                    op=ALU.mult,
                )
                if cols % 512 == 0:
                    for j in range(g0, g0 + cols, 512):
                        nc.tensor.matmul(
                            out=PACC_A[:],
                            lhsT=ONESB[:],
                            rhs=SQ[:, j : j + 512],
                            start=(k == a_idx[0] and j == g0),
                            stop=(k == a_idx[-1] and j == g0 + cols - 512),
                        )
                else:
                    nc.tensor.matmul(
                        out=PACC_B[:, 0:cols],
                        lhsT=ONESB[:],
                        rhs=SQ[:, g0 : g0 + cols],
                        start=(k == b_idx[0]),
                        stop=(k == b_idx[-1]),
                    )
                # emit bank readouts as soon as each bank completes so
                # they precede later tail work in the DVE engine FIFO
                if k == a_idx[-1]:
                    nc.vector.tensor_reduce(
                        out=OUTSB[0:1, 0:1], in_=PACC_A[:],
                        axis=mybir.AxisListType.X, op=ALU.add,
                    )
                if b_idx and k == b_idx[-1]:
                    nc.vector.tensor_reduce(
                        out=OUTSB[0:1, 1:2], in_=PACC_B[:, 0:b_cols],
                        axis=mybir.AxisListType.X, op=ALU.add,
                    )

            # ---- single 8B store; host sums the partials ----
            nc.sync.dma_start(out[:], OUTSB[:])

    nc.compile()
    return nc


def get_program():
    global _PROG
    if _PROG is None:
        _PROG = _build_program()
    return _PROG


def make_in_maps(policy_output, target_boxes=None, target_probs=None):
    policy_output = np.ascontiguousarray(
        np.asarray(policy_output, dtype=np.float32)
    )
    assert policy_output.shape == (B, C, H, W)
    return [{"pol": policy_output[i]} for i in range(N_CORES)]


def host_corr(pol_i, tb_i, tp_i):
    """Match-term correction (f64, <=48 anchors) from the full inputs.

    For each target box t and anchor a the corrected contribution replaces
    the dense fp term at that cell: coord + (conf-tp)^2 - conf^2
    = |pr-r2| + |pc-c2| + tp*(tp - 2*conf).
    """
    tbl = tb_i.astype(np.int64)
    g = pol_i[:, tbl[:, 0], tbl[:, 1]].astype(np.float64)  # [C, T]
    s = 1.0 / (1.0 + np.exp(-g))
    total = 0.0
    for t in range(T):
        if any((tbl[t] == tbl[t2]).all() for t2 in range(t)):
            continue  # an earlier identical box wins the match
        r, c, r2, c2 = (float(v) for v in tbl[t])
        tp = float(tp_i[t])
        for a in range(3):
            pr = min(max(r + 9.0 * s[3 * a + 0, t], 0.0), 511.0)
            pc = min(max(c + 16.0 * s[3 * a + 1, t], 0.0), 511.0)
            if np.round(pr) == r2 and np.round(pc) == c2:
                conf = s[3 * a + 2, t]
                total += abs(pr - r2) + abs(pc - c2) + tp * (tp - 2.0 * conf)
    return total


def kernel(policy_output, target_boxes, target_probs):
    from concourse.bass_utils import run_bass_kernel_spmd

    nc = get_program()
    pol = np.ascontiguousarray(np.asarray(policy_output, dtype=np.float32))
    tb = np.ascontiguousarray(np.asarray(target_boxes, dtype=np.int32))
    tp = np.ascontiguousarray(np.asarray(target_probs, dtype=np.float32))
    in_maps = make_in_maps(pol)
    res = None
    for attempt in range(3):
        try:
            res = run_bass_kernel_spmd(nc, in_maps, list(range(N_CORES)))
            break
        except Exception:
            # transient device/runtime hiccup: retry on a fresh attempt
            if attempt == 2:
                raise
    total = 0.0
    for i in range(N_CORES):
        total += float(res.results[i]["out"].sum(dtype=np.float64))
        total += host_corr(pol[i], tb[i], tp[i])
    return np.float32(total / DENOM)
